# revision 1
# baseline (speedup 1.0000x reference)
"""Trainium2 Bass kernel for tied-row MSA attention (nn_Attention_52329881535135).

Strategy (8 NeuronCores, one chip):
  - Shard the MSA row dim r (leading b*r=256) across the 8 cores: 32 rows each.
  - Each core: q/k projections for its rows into a transposed, row-pair-stacked
    per-head layout; the row-tied logits dotsT[h,j,i] = sum_r k_r^T q_r
    accumulate in PSUM (pairs fold into the 128-partition contraction).
  - Two bf16 AllReduces (heads 0-3 / 4-7) sum the logits over all 256 rows;
    the first overlaps the tail of phase 1 + v projections, the second
    overlaps softmax+attn@v+partial-output of the first head group.
  - Every core computes the replicated softmax (exp on ACT, column sums via
    ones-matmul, 1/sum folded into the exp tiles), then out = attn @ v for its
    own 32 rows in two head-group passes whose partial output projections
    accumulate into DRAM via CCE accumulate-DMA; host concatenates shards.

  Mask bookkeeping (has_rows / num_rows / mask_any) is computed on the host at
  call time and folded into the weights / an additive column bias, so the
  device graph only does dense matmuls.
"""

import sys

sys.path.insert(0, "/opt/trn_rl_repo")

import numpy as np

B, R, N, D, H, DH = 1, 256, 512, 256, 8, 64
INNER = H * DH
NCORES = 8
R_LOC = R // NCORES  # 32 rows per core
P = 128
NPT = N // P  # 4 position tiles
NJT = N // P  # 4 j tiles
NDT = D // P  # 2 d tiles
NHT = INNER // P  # 4 hd tiles
V_PREFETCH = 5  # pairs whose v-projection is emitted before softmax (AR overlap)
PASS_LA = 4  # pass-B lookahead (pairs of pass-A emitted before first pass-B)

_graph_cache = {}


def _build(
    separate_xq: bool,
    has_bias: bool = True,
    r_loc: int = R_LOC,
    n_cores: int = NCORES,
    do_finalize: bool = True,
):
    from contextlib import ExitStack

    from concourse import bacc, mybir, tile

    f32 = mybir.dt.float32
    bf16 = mybir.dt.bfloat16
    AF = mybir.ActivationFunctionType
    ALU = mybir.AluOpType

    nc = bacc.Bacc(
        "TRN2", target_bir_lowering=False, debug=False, num_devices=n_cores
    )

    x_ext = nc.declare_dram_parameter("x", [r_loc, N, D], f32, isOutput=False)
    if separate_xq:
        xq_ext = nc.declare_dram_parameter("xq", [r_loc, N, D], f32, isOutput=False)
    else:
        xq_ext = x_ext
    wq_ext = nc.declare_dram_parameter("Wq", [D, INNER], f32, isOutput=False)
    wk_ext = nc.declare_dram_parameter("Wk", [D, INNER], f32, isOutput=False)
    wv_ext = nc.declare_dram_parameter("Wv", [D, INNER], f32, isOutput=False)
    wo_ext = nc.declare_dram_parameter("Wo", [INNER, D], f32, isOutput=False)
    bo_ext = nc.declare_dram_parameter("bo", [D], f32, isOutput=False)
    jb_ext = nc.declare_dram_parameter("jbias", [NJT, P], f32, isOutput=False)
    out_ext = nc.declare_dram_parameter("out", [r_loc, N, D], f32, isOutput=True)

    # logits AllReduce split in two (heads 0..3 / 4..7), carried in bf16
    HH = H // 2
    cc_shape = [P, HH, NJT, N]
    out_space = "Shared" if n_cores > 4 else "Local"
    cc_in = [
        nc.dram_tensor("cc_in_a", cc_shape, bf16),
        nc.dram_tensor("cc_in_b", cc_shape, bf16),
    ]
    cc_out = [
        nc.dram_tensor("cc_out_a", cc_shape, bf16, addr_space=out_space),
        nc.dram_tensor("cc_out_b", cc_shape, bf16, addr_space=out_space),
    ]
    xbf_dram = nc.dram_tensor("xbf_dram", [r_loc, N, D], bf16)
    v2_dram = nc.dram_tensor("v2_dram", [r_loc // 2, P, NJT, H, 2, DH], bf16)
    if separate_xq:
        xqbf_dram = nc.dram_tensor("xqbf_dram", [r_loc, N, D], bf16)

    PAIRS = r_loc // 2
    groups = [list(range(g, min(g + 4, PAIRS))) for g in range(0, PAIRS, 4)]
    GMAX = max(len(g) for g in groups)

    with tile.TileContext(nc) as tc, ExitStack() as top:
        consts = top.enter_context(tc.tile_pool(name="consts", bufs=1))
        xt_pool = top.enter_context(tc.tile_pool(name="xt", bufs=2))
        # v pools live at top level so v-projections can run while phase-1
        # SBUF/PSUM (qk/dots) is still allocated, i.e. during the first AR
        v2_pool = top.enter_context(tc.tile_pool(name="v2p", bufs=5))
        vpsum = top.enter_context(tc.tile_pool(name="vpsum", bufs=2, space="PSUM"))

        # --- constants / weights (resident in SBUF, cast to bf16 on load) ---
        wq_sb = consts.tile([P, NDT, INNER], bf16, name="wq_sb")
        nc.gpsimd.dma_start(wq_sb[:], wq_ext.rearrange("(o p) f -> p o f", p=P))
        wk_sb = consts.tile([P, NDT, INNER], bf16, name="wk_sb")
        nc.gpsimd.dma_start(wk_sb[:], wk_ext.rearrange("(o p) f -> p o f", p=P))
        wv_sb = consts.tile([P, NDT, INNER], bf16, name="wv_sb")
        nc.gpsimd.dma_start(wv_sb[:], wv_ext.rearrange("(o p) f -> p o f", p=P))
        wo_sb = consts.tile([P, NHT, D], bf16, name="wo_sb")
        nc.gpsimd.dma_start(wo_sb[:], wo_ext.rearrange("(o p) e -> p o e", p=P))

        ones_col = consts.tile([P, 1], bf16, name="ones_col")
        nc.any.memset(ones_col, 1.0)
        ones_row = consts.tile([1, P], bf16, name="ones_row")
        nc.any.memset(ones_row, 1.0)
        jb_sb = consts.tile([P, NJT], f32, name="jb_sb")
        nc.sync.dma_start(jb_sb[:], jb_ext.rearrange("t p -> p t"))
        if has_bias:
            ones_row_f = consts.tile([1, P], f32, name="ones_row_f")
            nc.any.memset(ones_row_f, 1.0)
            bo_sb = consts.tile([1, D], f32, name="bo_sb")
            nc.sync.dma_start(bo_sb[:], bo_ext[None, :])
            bo_bcast = consts.tile([P, D], f32, name="bo_bcast")
            with tc.tile_pool(name="initpsum", bufs=1, space="PSUM") as initp:
                bp0 = initp.tile([P, D], f32, name="bp0")
                nc.tensor.matmul(
                    bp0[:], ones_row_f[:], bo_sb[:], start=True, stop=True
                )
                nc.any.tensor_copy(out=bo_bcast[:], in_=bp0[:])

        def cast_x(src_ext, dst_dram, r):
            nc.gpsimd.dma_start(dst_dram[r], src_ext[r])  # f32 -> bf16 cast DMA

        from concourse.tile_rust import add_dep_helper

        def load_xT(src_dram, r, tag, after=None):
            xT = xt_pool.tile([P, NDT, N], bf16, tag=f"xT_{tag}")
            for dh in range(NDT):
                tr = nc.sync.dma_start_transpose(
                    xT[:, dh, :], src_dram[r][:, dh * P : (dh + 1) * P]
                )
                if after is not None:
                    add_dep_helper(tr.ins, after, reason="delay into AR window")
            return xT

        def emit_v(pair, after=None, pool=None):
            """v projection for one row pair -> pair-stacked [j, (r0.hd|r1.hd)]."""
            v2 = (pool or v2_pool).tile(
                [P, NJT, H, 2, DH], bf16, tag="v2", name=f"v2_{pair}"
            )
            for parity in range(2):
                r = 2 * pair + parity
                xT = load_xT(xbf_dram, r, "p2", after=after)
                for pt in range(NPT):
                    vp = vpsum.tile([P, INNER], f32, tag="vp")
                    for dt in range(NDT):
                        nc.tensor.matmul(
                            vp[:],
                            xT[:, dt, pt * P : (pt + 1) * P],
                            wv_sb[:, dt, :],
                            start=(dt == 0),
                            stop=(dt == NDT - 1),
                        )
                    nc.scalar.copy(
                        v2[:, pt, :, parity, :],
                        vp.rearrange("p (h d) -> p h d", h=H),
                    )
            return v2

        # =====================  Phase 1: q/k + tied dots  =====================
        with ExitStack() as ph1:
            qk_pool = ph1.enter_context(tc.tile_pool(name="qk", bufs=1))
            dots_pool = ph1.enter_context(tc.tile_pool(name="dots", bufs=1))
            proj_psum = ph1.enter_context(
                tc.tile_pool(name="proj_psum", bufs=4, space="PSUM")
            )
            dots_psum = ph1.enter_context(
                tc.tile_pool(name="dots_psum", bufs=2, space="PSUM")
            )

            ccsb_pool = ph1.enter_context(tc.tile_pool(name="ccsb", bufs=8))
            dots_sb = dots_pool.tile([P, H, NJT, N], f32, name="dots_sb")
            flush_marker = [None]

            for gi, group in enumerate(groups):
                glen = len(group)
                q2 = qk_pool.tile([P, H, GMAX, N], bf16, tag="q2")
                k2 = qk_pool.tile([P, H, GMAX, N], bf16, tag="k2")
                for pq, pair in enumerate(group):
                    for parity in range(2):
                        r = 2 * pair + parity
                        cast_x(x_ext, xbf_dram, r)
                        xT = load_xT(xbf_dram, r, "p1")
                        if separate_xq:
                            cast_x(xq_ext, xqbf_dram, r)
                            xTq = load_xT(xqbf_dram, r, "p1q")
                        else:
                            xTq = xT
                        off = 64 * parity
                        for wsb, xtt, dest in (
                            (wq_sb, xTq, q2),
                            (wk_sb, xT, k2),
                        ):
                            for t in range(NHT):
                                pp = proj_psum.tile([P, N], f32, tag="pp")
                                for dt in range(NDT):
                                    nc.tensor.matmul(
                                        pp[:],
                                        wsb[:, dt, t * P : (t + 1) * P],
                                        xtt[:, dt, :],
                                        start=(dt == 0),
                                        stop=(dt == NDT - 1),
                                    )
                                # the two half-evacs go to different engines so
                                # the PSUM slot frees after ~one op latency
                                nc.vector.tensor_copy(
                                    dest[off : off + 64, 2 * t, pq, :], pp[0:64, :]
                                )
                                nc.scalar.copy(
                                    dest[off : off + 64, 2 * t + 1, pq, :],
                                    pp[64:128, :],
                                )
                # tied logits for this group: dotsT[h, j, i] += pair sums
                # (full 128-partition contraction = both rows of the pair)
                for h in range(H):
                    dps = {}
                    for jt in range(NJT):
                        dps[jt] = dots_psum.tile(
                            [P, N], f32, tag="dp", name=f"dp{h}_{jt}"
                        )
                        for pq in range(glen):
                            nc.tensor.matmul(
                                dps[jt][:],
                                k2[:, h, pq, jt * P : (jt + 1) * P],
                                q2[:, h, pq, :],
                                start=(pq == 0),
                                stop=(pq == glen - 1),
                            )
                    last = gi == len(groups) - 1
                    for jt in range(NJT):
                        if not last:
                            if gi == 0:
                                nc.vector.tensor_copy(
                                    dots_sb[:, h, jt, :], dps[jt][:]
                                )
                            else:
                                nc.vector.tensor_add(
                                    out=dots_sb[:, h, jt, :],
                                    in0=dps[jt][:],
                                    in1=dots_sb[:, h, jt, :],
                                )
                        else:
                            # final add emits the bf16 wire tile directly
                            cc_t = ccsb_pool.tile([P, N], bf16, tag="ccsb")
                            if len(groups) == 1:
                                flush_inst = nc.vector.tensor_copy(
                                    cc_t[:], dps[jt][:]
                                )
                            else:
                                flush_inst = nc.vector.tensor_add(
                                    out=cc_t[:],
                                    in0=dps[jt][:],
                                    in1=dots_sb[:, h, jt, :],
                                )
                            nc.sync.dma_start(
                                cc_in[0 if h < HH else 1][:, h % HH, jt, :],
                                cc_t[:],
                            )
                            if h == HH - 1 and jt == NJT - 1:
                                flush_marker[0] = flush_inst.ins
                    if last and h in (HH - 1, H - 1):
                        hg = 0 if h < HH else 1
                        nc.gpsimd.collective_compute(
                            "AllReduce",
                            ALU.add,
                            replica_groups=[list(range(n_cores))],
                            ins=[cc_in[hg][:]],
                            outs=[cc_out[hg][:]],
                        )

        # ============  Phase 2: v (overlaps AR1), softmax, attn, out  =========
        with ExitStack() as ph2:
            exp_pool = ph2.enter_context(tc.tile_pool(name="expp", bufs=1))
            rs_pool = ph2.enter_context(tc.tile_pool(name="rsp", bufs=1))
            dl_pool = ph2.enter_context(tc.tile_pool(name="dlp", bufs=2))
            sm_pool = ph2.enter_context(tc.tile_pool(name="smp", bufs=2))
            out2_pool = ph2.enter_context(tc.tile_pool(name="o2p", bufs=4))
            yrow_pool = ph2.enter_context(tc.tile_pool(name="yrow", bufs=3))
            attpsum = ph2.enter_context(
                tc.tile_pool(name="attpsum", bufs=6, space="PSUM")
            )

            exp_sb = exp_pool.tile([P, H, NJT, N], bf16, name="exp_sb")
            rs_bcast = rs_pool.tile([P, H, N], f32, name="rs_bcast")

            # v projections for the first pairs sit right after the dots
            # matmuls in the PE queue; with a dense phase 1 they land in the
            # AR1 window naturally (their xT loads prefetch during phase 1)
            v2s = {}
            for pair in range(min(V_PREFETCH, PAIRS)):
                v2s[pair] = emit_v(pair)

            # ---- softmax; 1/colsum is folded into the exp tiles ----
            def softmax_head(h):
                hg = 0 if h < HH else 1
                dl = dl_pool.tile([P, NJT, N], bf16, tag="dl")
                nc.scalar.dma_start(dl[:], cc_out[hg][:, h % HH, :, :])
                for jt in range(NJT):
                    nc.scalar.activation(
                        exp_sb[:, h, jt, :],
                        dl[:, jt, :],
                        AF.Exp,
                        bias=jb_sb[:, jt : jt + 1],
                        scale=1.0,
                    )
                sp = attpsum.tile([1, N], f32, tag="att")
                for jt in range(NJT):
                    nc.tensor.matmul(
                        sp[:],
                        ones_col[:],
                        exp_sb[:, h, jt, :],
                        start=(jt == 0),
                        stop=(jt == NJT - 1),
                    )
                s_bf = sm_pool.tile([1, N], bf16, tag="s_bf")
                nc.vector.tensor_copy(s_bf[:], sp[:])
                bps = attpsum.tile([P, N], f32, tag="att")
                nc.tensor.matmul(bps[:], ones_row[:], s_bf[:], start=True, stop=True)
                nc.vector.reciprocal(rs_bcast[:, h, :], bps[:])
                for jt in range(NJT):
                    nc.vector.tensor_tensor(
                        exp_sb[:, h, jt, :],
                        exp_sb[:, h, jt, :],
                        rs_bcast[:, h, :],
                        ALU.mult,
                    )

            for h in range(H):
                softmax_head(h)

            # ---- attn @ v + partial output projection, two head-group passes;
            # pass hg projects hd-tiles (2hg, 2hg+1) and accumulates into DRAM
            def attn_pass(pair, hg):
                v2 = v2s[pair]

                out2h = [
                    out2_pool.tile(
                        [P, 2, N], bf16, tag=f"o2_{hg}_{par}",
                        name=f"o2_{hg}_{par}_{pair}",
                    )
                    for par in range(2)
                ]
                for hh in range(HH):
                    h = hg * HH + hh
                    ap_ps = attpsum.tile([P, N], f32, tag="att")
                    for jt in range(NJT):
                        nc.tensor.matmul(
                            ap_ps[:],
                            v2[:, jt, h, :, :],
                            exp_sb[:, h, jt, :],
                            start=(jt == 0),
                            stop=(jt == NJT - 1),
                        )
                    t2, sub = hh // 2, hh % 2
                    for parity in range(2):
                        o = 64 * parity
                        evac = nc.vector.tensor_copy if parity == 0 else nc.scalar.copy
                        evac(
                            out2h[parity][sub * 64 : sub * 64 + 64, t2, :],
                            ap_ps[o : o + 64, :],
                        )
                for parity in range(2):
                    r = 2 * pair + parity
                    yrow = yrow_pool.tile([P, NPT, D], f32, tag="yrow")
                    for it in range(NPT):
                        yp = attpsum.tile([P, D], f32, tag="att")
                        for t2 in range(2):
                            nc.tensor.matmul(
                                yp[:],
                                out2h[parity][:, t2, it * P : (it + 1) * P],
                                wo_sb[:, 2 * hg + t2, :],
                                start=(t2 == 0),
                                stop=(t2 == 1),
                            )
                        if has_bias and hg == 1:
                            nc.vector.tensor_add(
                                out=yrow[:, it, :], in0=yp[:], in1=bo_bcast[:]
                            )
                        else:
                            nc.vector.tensor_copy(yrow[:, it, :], yp[:])
                    dst = out_ext[r].rearrange("(po pi) e -> pi po e", pi=P)
                    if hg == 0:
                        nc.sync.dma_start(dst, yrow[:])
                    else:
                        nc.gpsimd.dma_start(dst, yrow[:], accum_op=ALU.add)

            for i in range(PAIRS + PASS_LA):
                if i < PAIRS:
                    if i not in v2s:
                        v2s[i] = emit_v(i)
                    attn_pass(i, 0)
                if i >= PASS_LA:
                    attn_pass(i - PASS_LA, 1)
                    del v2s[i - PASS_LA]

    if do_finalize:
        nc.finalize()
    return nc


def _get_graph(separate_xq: bool, has_bias: bool):
    key = (separate_xq, has_bias)
    if key not in _graph_cache:
        _graph_cache[key] = _build(separate_xq, has_bias)
    return _graph_cache[key]


def _prepare(x, mask, Wq, Wk, Wv, Wo, bo, tie_attn_dim):
    """Host-side prep: mask bookkeeping, weight folding, sharded in_maps."""
    x = np.ascontiguousarray(np.asarray(x, dtype=np.float32))
    mask = np.asarray(mask).astype(bool)
    Wq = np.asarray(Wq, dtype=np.float32)
    Wk = np.ascontiguousarray(np.asarray(Wk, dtype=np.float32))
    Wv = np.ascontiguousarray(np.asarray(Wv, dtype=np.float32))
    Wo = np.ascontiguousarray(np.asarray(Wo, dtype=np.float32))
    bo = np.ascontiguousarray(np.asarray(bo, dtype=np.float32))
    r = int(tie_attn_dim)
    assert x.shape == (B * R, N, D) and r == R, (x.shape, r)

    m = mask.reshape(B, R, N)
    has_rows = m.any(axis=-1)[0]  # [R]
    num_rows = max(int(has_rows.sum()), 1)
    col_valid = m.any(axis=1)[0]  # [N]

    scale = (DH ** -0.5) * (num_rows ** -0.5)
    Wq_eff = np.ascontiguousarray(Wq * np.float32(scale))

    jbias = np.where(col_valid, 0.0, -1e30).astype(np.float32)
    jbias = np.ascontiguousarray(jbias.reshape(NJT, P))

    has_bias = bool(np.any(bo != 0.0))
    separate_xq = not bool(has_rows.all())
    if separate_xq:
        xq = np.ascontiguousarray(x * has_rows[:, None, None].astype(np.float32))
    else:
        xq = None

    in_maps = []
    for c in range(NCORES):
        im = {
            "x": np.ascontiguousarray(x[c * R_LOC : (c + 1) * R_LOC]),
            "Wq": Wq_eff,
            "Wk": Wk,
            "Wv": Wv,
            "Wo": Wo,
            "bo": bo,
            "jbias": jbias,
        }
        if separate_xq:
            im["xq"] = np.ascontiguousarray(xq[c * R_LOC : (c + 1) * R_LOC])
        in_maps.append(im)
    return separate_xq, has_bias, in_maps


def kernel(x, mask, Wq, Wk, Wv, Wo, bo, tie_attn_dim):
    from concourse.bass_utils import run_bass_kernel_spmd

    separate_xq, has_bias, in_maps = _prepare(
        x, mask, Wq, Wk, Wv, Wo, bo, tie_attn_dim
    )
    nc = _get_graph(separate_xq, has_bias)
    res = run_bass_kernel_spmd(nc, in_maps, list(range(NCORES)))
    out = np.concatenate([res.results[c]["out"] for c in range(NCORES)], axis=0)
    return out.astype(np.float32)


def _install_ntff_hook():
    """The agent image's antenv lacks axon_hooks; recreate it so trace=True
    can drive NTFF profiling through libaxon_pjrt.so (see trn_boot.py)."""
    try:
        from antenv import axon_hooks  # noqa: F401

        return
    except ImportError:
        pass
    import types

    import antenv

    mod = types.ModuleType("antenv.axon_hooks")
    holder = {}
    mod.set_axon_ntff_profile_hook = lambda h: holder.__setitem__("h", h)
    mod.get_axon_ntff_profile_hook = lambda: holder.get("h")
    sys.modules["antenv.axon_hooks"] = mod
    antenv.axon_hooks = mod
    if "/root/.axon_site" not in sys.path:
        sys.path.insert(0, "/root/.axon_site")
    from trn_agent_boot.trn_boot import _ntff_profile_via_ctypes

    mod.set_axon_ntff_profile_hook(
        _ntff_profile_via_ctypes("/opt/axon/libaxon_pjrt.so")
    )


def bench(inputs):
    """Run with neuron-profile tracing; returns (BassKernelResults, output)."""
    from concourse.bass_utils import run_bass_kernel_spmd

    _install_ntff_hook()
    separate_xq, has_bias, in_maps = _prepare(**inputs)
    nc = _get_graph(separate_xq, has_bias)
    res = run_bass_kernel_spmd(nc, in_maps, list(range(NCORES)), trace=True)
    out = np.concatenate([res.results[c]["out"] for c in range(NCORES)], axis=0)
    return res, out.astype(np.float32)



# revision 2
# speedup vs baseline: 1.2908x; 1.2908x over previous
"""Trainium2 Bass kernel for tied-row MSA attention (nn_Attention_52329881535135).

Strategy (8 NeuronCores, one chip):
  - Shard the MSA row dim r (leading b*r=256) across the 8 cores: 32 rows each.
  - The host pre-transposes and pre-casts x to bf16 tiles xT[r, dt, p, n]
    (d on partitions) so the device only does plain contiguous DMA loads
    (no f32->bf16 cast bounce, no DMA-transpose on the critical path).
  - Phase 1 is split into four head-pair quarters: each quarter projects
    q/k for heads (2g, 2g+1) for all 32 local rows (row pairs stacked into
    the 128-partition contraction), computes the row-tied logits
    dotsT[j, i] = sum_pairs k^T q in a single 16-matmul PSUM accumulation
    chain per (head, j-tile), and launches a 1MB bf16 AllReduce for its two
    heads.  All four AllReduces complete under the shadow of later compute.
  - Softmax for each head pair runs on ACT/DVE as soon as its AllReduce
    lands (exp with folded column-mask bias, column sums via ones-matmul,
    1/sum folded back into the exp tiles), overlapped with the next
    quarter's matmuls.
  - v projections + a single merged attention pass per row pair: attn @ v
    for all 8 heads, then the full output projection (accumulating all four
    hd-tiles in PSUM) and a single f32 store per row — no DRAM accumulate
    round trip.  Emitted in two half-batches of 8 pairs to bound SBUF.

  Mask bookkeeping (has_rows / num_rows / mask_any) is computed on the host
  at call time and folded into the weights / an additive column bias, so the
  device graph only does dense matmuls.
"""

import sys

sys.path.insert(0, "/opt/trn_rl_repo")

import numpy as np

B, R, N, D, H, DH = 1, 256, 512, 256, 8, 64
INNER = H * DH
NCORES = 8
R_LOC = R // NCORES  # 32 rows per core
P = 128
NPT = N // P  # 4 position tiles
NJT = N // P  # 4 j tiles
NDT = D // P  # 2 d tiles
NHT = INNER // P  # 4 hd tiles
PAIRS = R_LOC // 2  # 16 row pairs
HG = 4  # AllReduce chunks (one per head pair)
H_PER = H // HG  # 2 heads per chunk

_graph_cache = {}


def _build(
    separate_xq: bool,
    has_bias: bool = True,
    r_loc: int = R_LOC,
    n_cores: int = NCORES,
    do_finalize: bool = True,
):
    from contextlib import ExitStack

    from concourse import bacc, mybir, tile

    f32 = mybir.dt.float32
    bf16 = mybir.dt.bfloat16
    AF = mybir.ActivationFunctionType
    ALU = mybir.AluOpType

    pairs = r_loc // 2

    nc = bacc.Bacc(
        "TRN2", target_bir_lowering=False, debug=False, num_devices=n_cores
    )

    xt_ext = nc.declare_dram_parameter("xT", [r_loc, NDT, P, N], bf16, isOutput=False)
    if separate_xq:
        xqt_ext = nc.declare_dram_parameter(
            "xqT", [r_loc, NDT, P, N], bf16, isOutput=False
        )
    wq_ext = nc.declare_dram_parameter("Wq", [P, NDT, INNER], bf16, isOutput=False)
    wk_ext = nc.declare_dram_parameter("Wk", [P, NDT, INNER], bf16, isOutput=False)
    wv_ext = nc.declare_dram_parameter("Wv", [P, NDT, INNER], bf16, isOutput=False)
    wo_ext = nc.declare_dram_parameter("Wo", [P, NHT, D], bf16, isOutput=False)
    bo_ext = nc.declare_dram_parameter("bo", [D], f32, isOutput=False)
    jb_ext = nc.declare_dram_parameter("jbias", [NJT, P], f32, isOutput=False)
    out_ext = nc.declare_dram_parameter("out", [r_loc, N, D], f32, isOutput=True)

    cc_shape = [P, H_PER, NJT, N]
    out_space = "Shared" if n_cores > 4 else "Local"
    cc_in = [nc.dram_tensor(f"cc_in_{g}", cc_shape, bf16) for g in range(HG)]
    cc_out = [
        nc.dram_tensor(f"cc_out_{g}", cc_shape, bf16, addr_space=out_space)
        for g in range(HG)
    ]

    with tile.TileContext(nc) as tc, ExitStack() as top:
        consts = top.enter_context(tc.tile_pool(name="consts", bufs=1))
        exp_pool = top.enter_context(tc.tile_pool(name="expp", bufs=1))
        dl_pool = top.enter_context(tc.tile_pool(name="dlp", bufs=2))
        rs_pool = top.enter_context(tc.tile_pool(name="rsp", bufs=2))
        sm_pool = top.enter_context(tc.tile_pool(name="smp", bufs=2))
        smpsum = top.enter_context(tc.tile_pool(name="smpsum", bufs=2, space="PSUM"))

        # --- constants / weights (already bf16 + pre-rearranged on host) ---
        wv_sb = consts.tile([P, NDT, INNER], bf16, name="wv_sb")
        nc.sync.dma_start(wv_sb[:], wv_ext[:])
        wo_sb = consts.tile([P, NHT, D], bf16, name="wo_sb")
        nc.sync.dma_start(wo_sb[:], wo_ext[:])

        ones_col = consts.tile([P, 1], bf16, name="ones_col")
        nc.any.memset(ones_col, 1.0)
        ones_row = consts.tile([1, P], bf16, name="ones_row")
        nc.any.memset(ones_row, 1.0)
        jb_sb = consts.tile([P, NJT], f32, name="jb_sb")
        nc.sync.dma_start(jb_sb[:], jb_ext.rearrange("t p -> p t"))
        if has_bias:
            ones_row_f = consts.tile([1, P], f32, name="ones_row_f")
            nc.any.memset(ones_row_f, 1.0)
            bo_sb = consts.tile([1, D], f32, name="bo_sb")
            nc.sync.dma_start(bo_sb[:], bo_ext[None, :])
            bo_bcast = consts.tile([P, D], f32, name="bo_bcast")
            with tc.tile_pool(name="initpsum", bufs=1, space="PSUM") as initp:
                bp0 = initp.tile([P, D], f32, name="bp0")
                nc.tensor.matmul(
                    bp0[:], ones_row_f[:], bo_sb[:], start=True, stop=True
                )
                nc.any.tensor_copy(out=bo_bcast[:], in_=bp0[:])

        exp_sb = exp_pool.tile([P, H, NJT, N], bf16, name="exp_sb")

        # ---- softmax for one head; 1/colsum folded into the exp tiles ----
        def softmax_head(h):
            g, hh = h // H_PER, h % H_PER
            dl = dl_pool.tile([P, NJT, N], bf16, tag="dl", name=f"dl{h}")
            nc.sync.dma_start(dl[:], cc_out[g][:, hh, :, :])
            for jt in range(NJT):
                nc.scalar.activation(
                    exp_sb[:, h, jt, :],
                    dl[:, jt, :],
                    AF.Exp,
                    bias=jb_sb[:, jt : jt + 1],
                    scale=1.0,
                )
            sp = smpsum.tile([1, N], f32, tag="sm", name=f"sp{h}")
            for jt in range(NJT):
                nc.tensor.matmul(
                    sp[:],
                    ones_col[:],
                    exp_sb[:, h, jt, :],
                    start=(jt == 0),
                    stop=(jt == NJT - 1),
                )
            s_bf = sm_pool.tile([1, N], bf16, tag="s_bf", name=f"sbf{h}")
            nc.vector.tensor_copy(s_bf[:], sp[:])
            bps = smpsum.tile([P, N], f32, tag="sm", name=f"bps{h}")
            nc.tensor.matmul(bps[:], ones_row[:], s_bf[:], start=True, stop=True)
            rs = rs_pool.tile([P, N], f32, tag="rs", name=f"rs{h}")
            nc.vector.reciprocal(rs[:], bps[:])
            for jt in range(NJT):
                nc.vector.tensor_tensor(
                    exp_sb[:, h, jt, :],
                    exp_sb[:, h, jt, :],
                    rs[:],
                    ALU.mult,
                )

        # =========== Scope 1: four head-pair quarters of q/k + dots ==========
        with ExitStack() as sc1:
            wqk_pool = sc1.enter_context(tc.tile_pool(name="wqk", bufs=1))
            xt_pool = sc1.enter_context(tc.tile_pool(name="xt", bufs=1))
            ccsb_pool = sc1.enter_context(tc.tile_pool(name="ccsb", bufs=4))

            wq_sb = wqk_pool.tile([P, NDT, INNER], bf16, name="wq_sb")
            nc.sync.dma_start(wq_sb[:], wq_ext[:])
            wk_sb = wqk_pool.tile([P, NDT, INNER], bf16, name="wk_sb")
            nc.sync.dma_start(wk_sb[:], wk_ext[:])

            # resident x^T tiles (one per row; precise per-row deps + prefetch)
            xts = []
            xqts = []
            for r in range(r_loc):
                xt = xt_pool.tile([P, NDT, N], bf16, tag=f"xt{r}", name=f"xt{r}")
                nc.sync.dma_start(xt[:], xt_ext[r].rearrange("t p n -> p t n"))
                xts.append(xt)
                if separate_xq:
                    xqt = xt_pool.tile(
                        [P, NDT, N], bf16, tag=f"xqt{r}", name=f"xqt{r}"
                    )
                    nc.sync.dma_start(xqt[:], xqt_ext[r].rearrange("t p n -> p t n"))
                    xqts.append(xqt)
                else:
                    xqts.append(xt)

            for g in range(HG):
                with ExitStack() as ph:
                    qk_pool = ph.enter_context(tc.tile_pool(name=f"qk{g}", bufs=1))
                    pp_psum = ph.enter_context(
                        tc.tile_pool(name=f"pp{g}", bufs=4, space="PSUM")
                    )
                    dp_psum = ph.enter_context(
                        tc.tile_pool(name=f"dp{g}", bufs=2, space="PSUM")
                    )
                    q2 = qk_pool.tile([P, H_PER, pairs, N], bf16, name=f"q2_{g}")
                    k2 = qk_pool.tile([P, H_PER, pairs, N], bf16, name=f"k2_{g}")
                    for r in range(r_loc):
                        pair, parity = r // 2, r % 2
                        off = 64 * parity
                        for wsb, xtt, dest in (
                            (wq_sb, xqts[r], q2),
                            (wk_sb, xts[r], k2),
                        ):
                            pp = pp_psum.tile([P, N], f32, tag="pp")
                            for dt in range(NDT):
                                nc.tensor.matmul(
                                    pp[:],
                                    wsb[:, dt, g * P : (g + 1) * P],
                                    xtt[:, dt, :],
                                    start=(dt == 0),
                                    stop=(dt == NDT - 1),
                                )
                            # two half-evacs on different engines so the PSUM
                            # slot frees after ~one op latency
                            nc.vector.tensor_copy(
                                dest[off : off + 64, 0, pair, :], pp[0:64, :]
                            )
                            nc.scalar.copy(
                                dest[off : off + 64, 1, pair, :], pp[64:128, :]
                            )
                    # tied logits: one 16-matmul accumulation chain per (h, jt)
                    for hh in range(H_PER):
                        for jt in range(NJT):
                            dp = dp_psum.tile([P, N], f32, tag="dp")
                            for pq in range(pairs):
                                nc.tensor.matmul(
                                    dp[:],
                                    k2[:, hh, pq, jt * P : (jt + 1) * P],
                                    q2[:, hh, pq, :],
                                    start=(pq == 0),
                                    stop=(pq == pairs - 1),
                                )
                            cc_t = ccsb_pool.tile([P, N], bf16, tag="ccsb")
                            if (hh * NJT + jt) % 2 == 0:
                                nc.vector.tensor_copy(cc_t[:], dp[:])
                            else:
                                nc.scalar.copy(cc_t[:], dp[:])
                            nc.sync.dma_start(cc_in[g][:, hh, jt, :], cc_t[:])
                nc.gpsimd.collective_compute(
                    "AllReduce",
                    ALU.add,
                    replica_groups=[list(range(n_cores))],
                    ins=[cc_in[g][:]],
                    outs=[cc_out[g][:]],
                )
                # softmax for the previous quarter's heads (its AR has landed
                # by the time ACT drains this quarter's evacuations)
                if g >= 1:
                    softmax_head(2 * (g - 1))
                    softmax_head(2 * (g - 1) + 1)
            softmax_head(4)
            softmax_head(5)

        # ===== Scope 2: v projections + merged attn/out pass per row pair ====
        with ExitStack() as sc2:
            xt2_pool = sc2.enter_context(tc.tile_pool(name="xt2", bufs=4))
            v2_pool = sc2.enter_context(tc.tile_pool(name="v2p", bufs=10))
            vpsum = sc2.enter_context(tc.tile_pool(name="vpsum", bufs=2, space="PSUM"))
            out2_pool = sc2.enter_context(tc.tile_pool(name="o2p", bufs=3))
            yrow_pool = sc2.enter_context(tc.tile_pool(name="yrow", bufs=3))
            ap_psum = sc2.enter_context(tc.tile_pool(name="ap", bufs=2, space="PSUM"))
            yp_psum = sc2.enter_context(tc.tile_pool(name="yp", bufs=2, space="PSUM"))

            v2s = {}

            def emit_v(pair):
                v2 = v2_pool.tile(
                    [P, NJT, H, 2, DH], bf16, tag="v2", name=f"v2_{pair}"
                )
                for parity in range(2):
                    r = 2 * pair + parity
                    xt = xt2_pool.tile([P, NDT, N], bf16, tag="xt2", name=f"x2_{r}")
                    nc.sync.dma_start(xt[:], xt_ext[r].rearrange("t p n -> p t n"))
                    for pt in range(NPT):
                        vp = vpsum.tile([P, INNER], f32, tag="vp")
                        for dt in range(NDT):
                            nc.tensor.matmul(
                                vp[:],
                                xt[:, dt, pt * P : (pt + 1) * P],
                                wv_sb[:, dt, :],
                                start=(dt == 0),
                                stop=(dt == NDT - 1),
                            )
                        nc.scalar.copy(
                            v2[:, pt, :, parity, :],
                            vp.rearrange("p (h d) -> p h d", h=H),
                        )
                v2s[pair] = v2

            def attn_pair(pair):
                v2 = v2s.pop(pair)
                out2 = [
                    out2_pool.tile(
                        [P, NHT, N], bf16, tag=f"o2_{par}", name=f"o2_{par}_{pair}"
                    )
                    for par in range(2)
                ]
                for h in range(H):
                    ap = ap_psum.tile([P, N], f32, tag="ap")
                    for jt in range(NJT):
                        nc.tensor.matmul(
                            ap[:],
                            v2[:, jt, h, :, :],
                            exp_sb[:, h, jt, :],
                            start=(jt == 0),
                            stop=(jt == NJT - 1),
                        )
                    t2, sub = h // 2, h % 2
                    for par in range(2):
                        o = 64 * par
                        evac = nc.vector.tensor_copy if par == 0 else nc.scalar.copy
                        evac(
                            out2[par][sub * 64 : sub * 64 + 64, t2, :],
                            ap[o : o + 64, :],
                        )
                for par in range(2):
                    r = 2 * pair + par
                    yrow = yrow_pool.tile([P, NPT, D], f32, tag="yrow")
                    for it in range(NPT):
                        yp = yp_psum.tile([P, D], f32, tag="yp")
                        for t2 in range(NHT):
                            nc.tensor.matmul(
                                yp[:],
                                out2[par][:, t2, it * P : (it + 1) * P],
                                wo_sb[:, t2, :],
                                start=(t2 == 0),
                                stop=(t2 == NHT - 1),
                            )
                        if has_bias:
                            nc.vector.tensor_add(
                                out=yrow[:, it, :], in0=yp[:], in1=bo_bcast[:]
                            )
                        else:
                            nc.vector.tensor_copy(yrow[:, it, :], yp[:])
                    dst = out_ext[r].rearrange("(po pi) e -> pi po e", pi=P)
                    nc.gpsimd.dma_start(dst, yrow[:])

            half = pairs // 2
            for pair in range(half):
                emit_v(pair)
            softmax_head(6)
            softmax_head(7)
            for pair in range(half):
                attn_pair(pair)
            for pair in range(half, pairs):
                emit_v(pair)
            for pair in range(half, pairs):
                attn_pair(pair)

    if do_finalize:
        nc.finalize()
    return nc


def _get_graph(separate_xq: bool, has_bias: bool):
    key = (separate_xq, has_bias)
    if key not in _graph_cache:
        _graph_cache[key] = _build(separate_xq, has_bias)
    return _graph_cache[key]


def _prepare(x, mask, Wq, Wk, Wv, Wo, bo, tie_attn_dim):
    """Host-side prep: mask bookkeeping, weight folding, x transpose+cast,
    sharded in_maps."""
    import ml_dtypes

    bf = ml_dtypes.bfloat16

    x = np.asarray(x, dtype=np.float32)
    mask = np.asarray(mask).astype(bool)
    Wq = np.asarray(Wq, dtype=np.float32)
    Wk = np.asarray(Wk, dtype=np.float32)
    Wv = np.asarray(Wv, dtype=np.float32)
    Wo = np.asarray(Wo, dtype=np.float32)
    bo = np.ascontiguousarray(np.asarray(bo, dtype=np.float32))
    r = int(tie_attn_dim)
    assert x.shape == (B * R, N, D) and r == R, (x.shape, r)

    m = mask.reshape(B, R, N)
    has_rows = m.any(axis=-1)[0]  # [R]
    num_rows = max(int(has_rows.sum()), 1)
    col_valid = m.any(axis=1)[0]  # [N]

    scale = (DH ** -0.5) * (num_rows ** -0.5)
    Wq_eff = Wq * np.float32(scale)

    def prep_w(w):  # [D, INNER] -> [P, NDT, INNER] bf16
        return np.ascontiguousarray(
            w.reshape(NDT, P, -1).transpose(1, 0, 2).astype(bf)
        )

    wq_b = prep_w(Wq_eff)
    wk_b = prep_w(Wk)
    wv_b = prep_w(Wv)
    wo_b = np.ascontiguousarray(
        Wo.reshape(NHT, P, D).transpose(1, 0, 2).astype(bf)
    )

    jbias = np.where(col_valid, 0.0, -1e30).astype(np.float32)
    jbias = np.ascontiguousarray(jbias.reshape(NJT, P))

    has_bias = bool(np.any(bo != 0.0))
    separate_xq = not bool(has_rows.all())

    in_maps = []
    for c in range(NCORES):
        xs = x[c * R_LOC : (c + 1) * R_LOC]  # [r_loc, N, D]
        xT = np.ascontiguousarray(
            xs.transpose(0, 2, 1).reshape(R_LOC, NDT, P, N).astype(bf)
        )
        im = {
            "xT": xT,
            "Wq": wq_b,
            "Wk": wk_b,
            "Wv": wv_b,
            "Wo": wo_b,
            "bo": bo,
            "jbias": jbias,
        }
        if separate_xq:
            hr = has_rows[c * R_LOC : (c + 1) * R_LOC].astype(np.float32)
            xq = xs * hr[:, None, None]
            im["xqT"] = np.ascontiguousarray(
                xq.transpose(0, 2, 1).reshape(R_LOC, NDT, P, N).astype(bf)
            )
        in_maps.append(im)
    return separate_xq, has_bias, in_maps


def kernel(x, mask, Wq, Wk, Wv, Wo, bo, tie_attn_dim):
    from concourse.bass_utils import run_bass_kernel_spmd

    separate_xq, has_bias, in_maps = _prepare(
        x, mask, Wq, Wk, Wv, Wo, bo, tie_attn_dim
    )
    nc = _get_graph(separate_xq, has_bias)
    res = run_bass_kernel_spmd(nc, in_maps, list(range(NCORES)))
    out = np.concatenate([res.results[c]["out"] for c in range(NCORES)], axis=0)
    return out.astype(np.float32)


def _install_ntff_hook():
    """The agent image's antenv lacks axon_hooks; recreate it so trace=True
    can drive NTFF profiling through libaxon_pjrt.so (see trn_boot.py)."""
    try:
        from antenv import axon_hooks  # noqa: F401

        return
    except ImportError:
        pass
    import types

    import antenv

    mod = types.ModuleType("antenv.axon_hooks")
    holder = {}
    mod.set_axon_ntff_profile_hook = lambda h: holder.__setitem__("h", h)
    mod.get_axon_ntff_profile_hook = lambda: holder.get("h")
    sys.modules["antenv.axon_hooks"] = mod
    antenv.axon_hooks = mod
    if "/root/.axon_site" not in sys.path:
        sys.path.insert(0, "/root/.axon_site")
    from trn_agent_boot.trn_boot import _ntff_profile_via_ctypes

    mod.set_axon_ntff_profile_hook(
        _ntff_profile_via_ctypes("/opt/axon/libaxon_pjrt.so")
    )


def bench(inputs):
    """Run with neuron-profile tracing; returns (BassKernelResults, output)."""
    from concourse.bass_utils import run_bass_kernel_spmd

    _install_ntff_hook()
    separate_xq, has_bias, in_maps = _prepare(**inputs)
    nc = _get_graph(separate_xq, has_bias)
    res = run_bass_kernel_spmd(nc, in_maps, list(range(NCORES)), trace=True)
    out = np.concatenate([res.results[c]["out"] for c in range(NCORES)], axis=0)
    return res, out.astype(np.float32)


# revision 7
# speedup vs baseline: 1.3548x; 1.0496x over previous
"""Trainium2 Bass kernel for tied-row MSA attention (nn_Attention_52329881535135).

Strategy (8 NeuronCores, one chip):
  - Shard the MSA row dim r (leading b*r=256) across the 8 cores: 32 rows each.
  - The host pre-transposes and pre-casts x to bf16 tiles xT[r, dt, p, n]
    (d on partitions) so the device only does plain contiguous DMA loads
    (no f32->bf16 cast bounce, no DMA-transpose on the critical path).
  - Phase 1 is split into four head-pair quarters: each quarter projects
    q/k for heads (2g, 2g+1) for all 32 local rows (row pairs stacked into
    the 128-partition contraction), computes the row-tied logits
    dotsT[j, i] = sum_pairs k^T q in a single 16-matmul PSUM accumulation
    chain per (head, j-tile), and launches a 1MB bf16 AllReduce for its two
    heads.  All four AllReduces complete under the shadow of later compute.
  - Softmax for each head pair runs on ACT/DVE as soon as its AllReduce
    lands (exp with folded column-mask bias, column sums via ones-matmul,
    1/sum folded back into the exp tiles), overlapped with the next
    quarter's matmuls.
  - v projections + a single merged attention pass per row pair: attn @ v
    for all 8 heads, then the full output projection (accumulating all four
    hd-tiles in PSUM) and a single f32 store per row — no DRAM accumulate
    round trip.  Emitted in two half-batches of 8 pairs to bound SBUF.

  Mask bookkeeping (has_rows / num_rows / mask_any) is computed on the host
  at call time and folded into the weights / an additive column bias, so the
  device graph only does dense matmuls.
"""

import sys

sys.path.insert(0, "/opt/trn_rl_repo")

import numpy as np

B, R, N, D, H, DH = 1, 256, 512, 256, 8, 64
INNER = H * DH
NCORES = 8
R_LOC = R // NCORES  # 32 rows per core
P = 128
NPT = N // P  # 4 position tiles
NJT = N // P  # 4 j tiles
NDT = D // P  # 2 d tiles
NHT = INNER // P  # 4 hd tiles
PAIRS = R_LOC // 2  # 16 row pairs
HG = 4  # AllReduce chunks (one per head pair)
H_PER = H // HG  # 2 heads per chunk

_graph_cache = {}


def _build(
    separate_xq: bool,
    has_bias: bool = True,
    r_loc: int = R_LOC,
    n_cores: int = NCORES,
    do_finalize: bool = True,
):
    from contextlib import ExitStack

    from concourse import bacc, mybir, tile

    f32 = mybir.dt.float32
    bf16 = mybir.dt.bfloat16
    AF = mybir.ActivationFunctionType
    ALU = mybir.AluOpType

    pairs = r_loc // 2

    nc = bacc.Bacc(
        "TRN2", target_bir_lowering=False, debug=False, num_devices=n_cores
    )

    xt_ext = nc.declare_dram_parameter("xT", [r_loc, NDT, P, N], bf16, isOutput=False)
    if separate_xq:
        xqt_ext = nc.declare_dram_parameter(
            "xqT", [r_loc, NDT, P, N], bf16, isOutput=False
        )
    wq_ext = nc.declare_dram_parameter("Wq", [P, NDT, INNER], bf16, isOutput=False)
    wk_ext = nc.declare_dram_parameter("Wk", [P, NDT, INNER], bf16, isOutput=False)
    wv_ext = nc.declare_dram_parameter("Wv", [P, NDT, INNER], bf16, isOutput=False)
    wo_ext = nc.declare_dram_parameter("Wo", [P, NHT, D], bf16, isOutput=False)
    bo_ext = nc.declare_dram_parameter("bo", [D], f32, isOutput=False)
    jb_ext = nc.declare_dram_parameter("jbias", [NJT, P], f32, isOutput=False)
    out_ext = nc.declare_dram_parameter("out", [r_loc, N, D], f32, isOutput=True)

    cc_shape = [P, H_PER, NJT, N]
    out_space = "Shared" if n_cores > 4 else "Local"
    cc_in = [nc.dram_tensor(f"cc_in_{g}", cc_shape, bf16) for g in range(HG)]
    cc_out = [
        nc.dram_tensor(f"cc_out_{g}", cc_shape, bf16, addr_space=out_space)
        for g in range(HG)
    ]

    with tile.TileContext(nc) as tc, ExitStack() as top:
        consts = top.enter_context(tc.tile_pool(name="consts", bufs=1))
        exp_pool = top.enter_context(tc.tile_pool(name="expp", bufs=1))
        dl_pool = top.enter_context(tc.tile_pool(name="dlp", bufs=2))
        rs_pool = top.enter_context(tc.tile_pool(name="rsp", bufs=2))
        sm_pool = top.enter_context(tc.tile_pool(name="smp", bufs=2))
        smpsum = top.enter_context(tc.tile_pool(name="smpsum", bufs=2, space="PSUM"))

        # --- constants / weights (already bf16 + pre-rearranged on host) ---
        wv_sb = consts.tile([P, NDT, INNER], bf16, name="wv_sb")
        nc.sync.dma_start(wv_sb[:], wv_ext[:])
        wo_sb = consts.tile([P, NHT, D], bf16, name="wo_sb")
        nc.sync.dma_start(wo_sb[:], wo_ext[:])

        ones_col = consts.tile([P, 1], bf16, name="ones_col")
        nc.any.memset(ones_col, 1.0)
        ones_row = consts.tile([1, P], bf16, name="ones_row")
        nc.any.memset(ones_row, 1.0)
        jb_sb = consts.tile([P, NJT], f32, name="jb_sb")
        nc.sync.dma_start(jb_sb[:], jb_ext.rearrange("t p -> p t"))
        if has_bias:
            ones_row_f = consts.tile([1, P], f32, name="ones_row_f")
            nc.any.memset(ones_row_f, 1.0)
            bo_sb = consts.tile([1, D], f32, name="bo_sb")
            nc.sync.dma_start(bo_sb[:], bo_ext[None, :])
            bo_bcast = consts.tile([P, D], f32, name="bo_bcast")
            with tc.tile_pool(name="initpsum", bufs=1, space="PSUM") as initp:
                bp0 = initp.tile([P, D], f32, name="bp0")
                nc.tensor.matmul(
                    bp0[:], ones_row_f[:], bo_sb[:], start=True, stop=True
                )
                nc.any.tensor_copy(out=bo_bcast[:], in_=bp0[:])

        exp_sb = exp_pool.tile([P, H, NJT, N], bf16, name="exp_sb")

        from concourse.tile_rust import add_dep_helper

        # ---- softmax, two emission halves so neither the ACT queue nor the
        # PE queue ever head-of-line blocks on an un-landed AllReduce ----
        def softmax_load_exp(h, after=None):
            g, hh = h // H_PER, h % H_PER
            dl = dl_pool.tile([P, NJT, N], bf16, tag="dl", name=f"dl{h}")
            tr = nc.sync.dma_start(dl[:], cc_out[g][:, hh, :, :])
            if after is not None:
                add_dep_helper(tr.ins, after, reason="hold exp until AR window")
            for jt in range(NJT):
                nc.scalar.activation(
                    exp_sb[:, h, jt, :],
                    dl[:, jt, :],
                    AF.Exp,
                    bias=jb_sb[:, jt : jt + 1],
                    scale=1.0,
                )

        def softmax_norm(h):
            sp = smpsum.tile([1, N], f32, tag="sm", name=f"sp{h}")
            for jt in range(NJT):
                nc.tensor.matmul(
                    sp[:],
                    ones_col[:],
                    exp_sb[:, h, jt, :],
                    start=(jt == 0),
                    stop=(jt == NJT - 1),
                )
            s_bf = sm_pool.tile([1, N], bf16, tag="s_bf", name=f"sbf{h}")
            nc.vector.tensor_copy(s_bf[:], sp[:])
            bps = smpsum.tile([P, N], f32, tag="sm", name=f"bps{h}")
            nc.tensor.matmul(bps[:], ones_row[:], s_bf[:], start=True, stop=True)
            rs = rs_pool.tile([P, N], f32, tag="rs", name=f"rs{h}")
            nc.vector.reciprocal_approx_fast(rs[:], bps[:])
            for jt in range(NJT):
                nc.vector.tensor_tensor(
                    exp_sb[:, h, jt, :],
                    exp_sb[:, h, jt, :],
                    rs[:],
                    ALU.mult,
                )

        # =========== Scope 1: four head-pair quarters of q/k + dots ==========
        with ExitStack() as sc1:
            wqk_pool = sc1.enter_context(tc.tile_pool(name="wqk", bufs=1))
            xt_pool = sc1.enter_context(tc.tile_pool(name="xt", bufs=1))
            ccsb_pool = sc1.enter_context(tc.tile_pool(name="ccsb", bufs=4))

            wq_sb = wqk_pool.tile([P, NDT, INNER], bf16, name="wq_sb")
            nc.sync.dma_start(wq_sb[:], wq_ext[:])
            wk_sb = wqk_pool.tile([P, NDT, INNER], bf16, name="wk_sb")
            nc.sync.dma_start(wk_sb[:], wk_ext[:])

            # resident x^T tiles (one per row; precise per-row deps + prefetch)
            xts = []
            xqts = []
            for r in range(r_loc):
                xt = xt_pool.tile([P, NDT, N], bf16, tag=f"xt{r}", name=f"xt{r}")
                nc.sync.dma_start(xt[:], xt_ext[r].rearrange("t p n -> p t n"))
                xts.append(xt)
                if separate_xq:
                    xqt = xt_pool.tile(
                        [P, NDT, N], bf16, tag=f"xqt{r}", name=f"xqt{r}"
                    )
                    nc.sync.dma_start(xqt[:], xqt_ext[r].rearrange("t p n -> p t n"))
                    xqts.append(xqt)
                else:
                    xqts.append(xt)

            dots_marker = [None] * HG
            for g in range(HG):
                with ExitStack() as ph:
                    qk_pool = ph.enter_context(tc.tile_pool(name=f"qk{g}", bufs=1))
                    pp_psum = ph.enter_context(
                        tc.tile_pool(name=f"pp{g}", bufs=4, space="PSUM")
                    )
                    dp_psum = ph.enter_context(
                        tc.tile_pool(name=f"dp{g}", bufs=2, space="PSUM")
                    )
                    q2 = qk_pool.tile([P, H_PER, pairs, N], bf16, name=f"q2_{g}")
                    k2 = qk_pool.tile([P, H_PER, pairs, N], bf16, name=f"k2_{g}")
                    for r in range(r_loc):
                        # normalization chain for the heads two quarters back,
                        # mid-quarter: their exps are long done, so the small
                        # colsum matmuls never stall the PE queue
                        if r == r_loc // 2 and g >= 2:
                            softmax_norm(2 * (g - 2))
                            softmax_norm(2 * (g - 2) + 1)
                        pair, parity = r // 2, r % 2
                        off = 64 * parity
                        for wsb, xtt, dest in (
                            (wq_sb, xqts[r], q2),
                            (wk_sb, xts[r], k2),
                        ):
                            pp = pp_psum.tile([P, N], f32, tag="pp")
                            for dt in range(NDT):
                                nc.tensor.matmul(
                                    pp[:],
                                    wsb[:, dt, g * P : (g + 1) * P],
                                    xtt[:, dt, :],
                                    start=(dt == 0),
                                    stop=(dt == NDT - 1),
                                )
                            # two half-evacs on different engines so the PSUM
                            # slot frees after ~one op latency
                            nc.vector.tensor_copy(
                                dest[off : off + 64, 0, pair, :], pp[0:64, :]
                            )
                            nc.scalar.copy(
                                dest[off : off + 64, 1, pair, :], pp[64:128, :]
                            )
                    # tied logits: one 16-matmul accumulation chain per (h, jt)
                    for hh in range(H_PER):
                        for jt in range(NJT):
                            dp = dp_psum.tile([P, N], f32, tag="dp")
                            for pq in range(pairs):
                                nc.tensor.matmul(
                                    dp[:],
                                    k2[:, hh, pq, jt * P : (jt + 1) * P],
                                    q2[:, hh, pq, :],
                                    start=(pq == 0),
                                    stop=(pq == pairs - 1),
                                )
                            cc_t = ccsb_pool.tile([P, N], bf16, tag="ccsb")
                            if (hh * NJT + jt) % 2 == 0:
                                ev = nc.vector.tensor_copy(cc_t[:], dp[:])
                            else:
                                ev = nc.scalar.copy(cc_t[:], dp[:])
                            nc.sync.dma_start(cc_in[g][:, hh, jt, :], cc_t[:])
                    dots_marker[g] = ev.ins
                nc.gpsimd.collective_compute(
                    "AllReduce",
                    ALU.add,
                    replica_groups=[list(range(n_cores))],
                    ins=[cc_in[g][:]],
                    outs=[cc_out[g][:]],
                )
                # exp for the previous quarter's heads: its AR lands while this
                # quarter computes; the dep on this quarter's last dots-evac
                # keeps the scheduler from hoisting the exps ahead of
                # independent evacuations in the strict-FIFO ACT queue
                if g >= 1:
                    softmax_load_exp(2 * (g - 1), after=dots_marker[g])
                    softmax_load_exp(2 * (g - 1) + 1, after=dots_marker[g])
            softmax_load_exp(4, after=dots_marker[3])
            softmax_load_exp(5, after=dots_marker[3])

        # ===== Scope 2: v projections + merged attn/out pass per row pair ====
        with ExitStack() as sc2:
            xt2_pool = sc2.enter_context(tc.tile_pool(name="xt2", bufs=4))
            v2_pool = sc2.enter_context(tc.tile_pool(name="v2p", bufs=13))
            vpsum = sc2.enter_context(tc.tile_pool(name="vpsum", bufs=2, space="PSUM"))
            out2_pool = sc2.enter_context(tc.tile_pool(name="o2p", bufs=3))
            yrow_pool = sc2.enter_context(tc.tile_pool(name="yrow", bufs=3))
            ap_psum = sc2.enter_context(tc.tile_pool(name="ap", bufs=2, space="PSUM"))
            yp_psum = sc2.enter_context(tc.tile_pool(name="yp", bufs=2, space="PSUM"))

            v2s = {}

            def emit_v(pair):
                v2 = v2_pool.tile(
                    [P, NJT, H, 2, DH], bf16, tag="v2", name=f"v2_{pair}"
                )
                ev = None
                for parity in range(2):
                    r = 2 * pair + parity
                    xt = xt2_pool.tile([P, NDT, N], bf16, tag="xt2", name=f"x2_{r}")
                    nc.sync.dma_start(xt[:], xt_ext[r].rearrange("t p n -> p t n"))
                    for pt in range(NPT):
                        vp = vpsum.tile([P, INNER], f32, tag="vp")
                        for dt in range(NDT):
                            nc.tensor.matmul(
                                vp[:],
                                xt[:, dt, pt * P : (pt + 1) * P],
                                wv_sb[:, dt, :],
                                start=(dt == 0),
                                stop=(dt == NDT - 1),
                            )
                        ev = nc.scalar.copy(
                            v2[:, pt, :, parity, :],
                            vp.rearrange("p (h d) -> p h d", h=H),
                        )
                v2s[pair] = v2
                return ev.ins

            def attn_pair(pair):
                v2 = v2s.pop(pair)
                out2 = [
                    out2_pool.tile(
                        [P, NHT, N], bf16, tag=f"o2_{par}", name=f"o2_{par}_{pair}"
                    )
                    for par in range(2)
                ]
                for h in range(H):
                    ap = ap_psum.tile([P, N], f32, tag="ap")
                    for jt in range(NJT):
                        nc.tensor.matmul(
                            ap[:],
                            v2[:, jt, h, :, :],
                            exp_sb[:, h, jt, :],
                            start=(jt == 0),
                            stop=(jt == NJT - 1),
                        )
                    t2, sub = h // 2, h % 2
                    for par in range(2):
                        o = 64 * par
                        evac = nc.vector.tensor_copy if par == 0 else nc.scalar.copy
                        evac(
                            out2[par][sub * 64 : sub * 64 + 64, t2, :],
                            ap[o : o + 64, :],
                        )
                for par in range(2):
                    r = 2 * pair + par
                    yrow = yrow_pool.tile([P, NPT, D], f32, tag="yrow")
                    for it in range(NPT):
                        yp = yp_psum.tile([P, D], f32, tag="yp")
                        for t2 in range(NHT):
                            nc.tensor.matmul(
                                yp[:],
                                out2[par][:, t2, it * P : (it + 1) * P],
                                wo_sb[:, t2, :],
                                start=(t2 == 0),
                                stop=(t2 == NHT - 1),
                            )
                        if has_bias:
                            nc.vector.tensor_add(
                                out=yrow[:, it, :], in0=yp[:], in1=bo_bcast[:]
                            )
                        else:
                            nc.vector.tensor_copy(yrow[:, it, :], yp[:])
                    dst = out_ext[r].rearrange("(po pi) e -> pi po e", pi=P)
                    nc.gpsimd.dma_start(dst, yrow[:])

            # 12 v-projections up front (the last AR + exp h6/h7 land under
            # them), then attn pairs with the remaining v-projections woven in
            lead = 12
            for pair in range(lead):
                vm = emit_v(pair)
                if pair == 1:
                    softmax_norm(4)
                    softmax_norm(5)
                elif pair == 7:
                    softmax_load_exp(6, after=vm)
                    softmax_load_exp(7, after=vm)
                elif pair == 9:
                    softmax_norm(6)
                    softmax_norm(7)
            for i, pair in enumerate(range(lead, pairs)):
                attn_pair(i)
                emit_v(pair)
            for i in range(pairs - lead, pairs):
                attn_pair(i)

    if do_finalize:
        nc.finalize()
    return nc


def _get_graph(separate_xq: bool, has_bias: bool):
    key = (separate_xq, has_bias)
    if key not in _graph_cache:
        _graph_cache[key] = _build(separate_xq, has_bias)
    return _graph_cache[key]


def _prepare(x, mask, Wq, Wk, Wv, Wo, bo, tie_attn_dim):
    """Host-side prep: mask bookkeeping, weight folding, x transpose+cast,
    sharded in_maps."""
    import ml_dtypes

    bf = ml_dtypes.bfloat16

    x = np.asarray(x, dtype=np.float32)
    mask = np.asarray(mask).astype(bool)
    Wq = np.asarray(Wq, dtype=np.float32)
    Wk = np.asarray(Wk, dtype=np.float32)
    Wv = np.asarray(Wv, dtype=np.float32)
    Wo = np.asarray(Wo, dtype=np.float32)
    bo = np.ascontiguousarray(np.asarray(bo, dtype=np.float32))
    r = int(tie_attn_dim)
    assert x.shape == (B * R, N, D) and r == R, (x.shape, r)

    m = mask.reshape(B, R, N)
    has_rows = m.any(axis=-1)[0]  # [R]
    num_rows = max(int(has_rows.sum()), 1)
    col_valid = m.any(axis=1)[0]  # [N]

    scale = (DH ** -0.5) * (num_rows ** -0.5)
    Wq_eff = Wq * np.float32(scale)

    def prep_w(w):  # [D, INNER] -> [P, NDT, INNER] bf16
        return np.ascontiguousarray(
            w.reshape(NDT, P, -1).transpose(1, 0, 2).astype(bf)
        )

    wq_b = prep_w(Wq_eff)
    wk_b = prep_w(Wk)
    wv_b = prep_w(Wv)
    wo_b = np.ascontiguousarray(
        Wo.reshape(NHT, P, D).transpose(1, 0, 2).astype(bf)
    )

    jbias = np.where(col_valid, 0.0, -1e30).astype(np.float32)
    jbias = np.ascontiguousarray(jbias.reshape(NJT, P))

    has_bias = bool(np.any(bo != 0.0))
    separate_xq = not bool(has_rows.all())

    in_maps = []
    for c in range(NCORES):
        xs = x[c * R_LOC : (c + 1) * R_LOC]  # [r_loc, N, D]
        xT = np.ascontiguousarray(
            xs.transpose(0, 2, 1).reshape(R_LOC, NDT, P, N).astype(bf)
        )
        im = {
            "xT": xT,
            "Wq": wq_b,
            "Wk": wk_b,
            "Wv": wv_b,
            "Wo": wo_b,
            "bo": bo,
            "jbias": jbias,
        }
        if separate_xq:
            hr = has_rows[c * R_LOC : (c + 1) * R_LOC].astype(np.float32)
            xq = xs * hr[:, None, None]
            im["xqT"] = np.ascontiguousarray(
                xq.transpose(0, 2, 1).reshape(R_LOC, NDT, P, N).astype(bf)
            )
        in_maps.append(im)
    return separate_xq, has_bias, in_maps


def kernel(x, mask, Wq, Wk, Wv, Wo, bo, tie_attn_dim):
    from concourse.bass_utils import run_bass_kernel_spmd

    separate_xq, has_bias, in_maps = _prepare(
        x, mask, Wq, Wk, Wv, Wo, bo, tie_attn_dim
    )
    nc = _get_graph(separate_xq, has_bias)
    res = run_bass_kernel_spmd(nc, in_maps, list(range(NCORES)))
    out = np.concatenate([res.results[c]["out"] for c in range(NCORES)], axis=0)
    return out.astype(np.float32)


def _install_ntff_hook():
    """The agent image's antenv lacks axon_hooks; recreate it so trace=True
    can drive NTFF profiling through libaxon_pjrt.so (see trn_boot.py)."""
    try:
        from antenv import axon_hooks  # noqa: F401

        return
    except ImportError:
        pass
    import types

    import antenv

    mod = types.ModuleType("antenv.axon_hooks")
    holder = {}
    mod.set_axon_ntff_profile_hook = lambda h: holder.__setitem__("h", h)
    mod.get_axon_ntff_profile_hook = lambda: holder.get("h")
    sys.modules["antenv.axon_hooks"] = mod
    antenv.axon_hooks = mod
    if "/root/.axon_site" not in sys.path:
        sys.path.insert(0, "/root/.axon_site")
    from trn_agent_boot.trn_boot import _ntff_profile_via_ctypes

    mod.set_axon_ntff_profile_hook(
        _ntff_profile_via_ctypes("/opt/axon/libaxon_pjrt.so")
    )


def bench(inputs):
    """Run with neuron-profile tracing; returns (BassKernelResults, output)."""
    from concourse.bass_utils import run_bass_kernel_spmd

    _install_ntff_hook()
    separate_xq, has_bias, in_maps = _prepare(**inputs)
    nc = _get_graph(separate_xq, has_bias)
    res = run_bass_kernel_spmd(nc, in_maps, list(range(NCORES)), trace=True)
    out = np.concatenate([res.results[c]["out"] for c in range(NCORES)], axis=0)
    return res, out.astype(np.float32)
